# revision 5
# baseline (speedup 1.0000x reference)
"""Trainium2 Bass kernel for a 2-layer GCN + edge-pair MLP head (8 NeuronCores).

Strategy (graph-parallel, per sharding hint):
  - Nodes partitioned contiguously across 8 cores (6250/core, padded to 6272).
  - Per-layer: each core computes xw = x_shard @ W (pre-scaled by dinv and with
    BN scale folded into W), all-gathers the per-node message table in two
    halves (A/B), then aggregates its incident edges (bucketed by dst block and
    src half) with dma_gather + a two-level segmented-sum matmul scheme:
      level 1: constant block-diagonal [128,32] matmul sums groups of 4 slots
      level 2: per-chunk one-hot matmul scatters 128 group-sums into the
               128-node dst block psum.
  - Head: per-node AB table (z @ [hW1_top | hW1_bot]), all-gathered, then
    per-pair gathers + elementwise relu/dot/sigmoid.
All indices / bucketing / padding are prepared host-side (numpy); all FLOPs run
on device. Tables are split into two halves so dma_gather's int16 indices
cover all rows, and so collectives overlap edge processing.
"""
import numpy as np
import ml_dtypes

import concourse.bass as bass
import concourse.bacc as bacc
import concourse.mybir as mybir
import concourse.tile as tile
from concourse.bass_utils import run_bass_kernel_spmd

F32 = mybir.dt.float32
BF16 = mybir.dt.bfloat16
I16 = mybir.dt.int16

NC = 8
N, E, P = 50000, 1600000, 500000
IN_DIM, HID, OUT = 512, 128, 64
EPS = 1e-5

NPC = 6250                 # real nodes per core
LPC = 6272                 # padded local nodes (49 blocks)
NBLK = 49
HLOC_A = 3200              # local rows in half A (25 blocks)
HLOC_B = LPC - HLOC_A      # 3072 (24 blocks)
HPA = HLOC_A + 16          # bounce stripe incl. 16 zero pad rows
HPB = HLOC_B + 16
NBLK_A = 25
VA = NC * HPA              # table-A rows (8 stripes)
VB = NC * HPB
ZA, ZB = HLOC_A, HLOC_B    # zero-row: core 0's first pad row
GS = 4                     # level-1 group size
CHUNK = 128 * GS           # slots per level-2 chunk
WCH = 16                   # chunks per gather window (8192 slots)
HWCH = 32                  # head chunks per window (5120 pairs)
PPC = P // NC              # pairs per core
GQ = 4                     # gather queue rotation

_CACHE = {}


def _node_half_row(u):
    """Global node id -> (half, half-table row). Vectorized."""
    cu = u // NPC
    iu = u - cu * NPC
    ha = iu < HLOC_A
    row = np.where(ha, cu * HPA + iu, cu * HPB + (iu - HLOC_A))
    return (~ha).astype(np.int64), row


def _wrap_idx(slots):
    """int64 row ids -> dma_gather int16 wrapped+replicated [128, S/16]."""
    s = np.asarray(slots, np.int16)
    assert len(s) % 16 == 0
    w = s.reshape(-1, 16).T
    return np.tile(w, (8, 1)).copy()


def _prep_edges(ei):
    """Bucket edges by (core, src-half, dst-block); build slot/index arrays.

    Returns per-core dicts + shared structure (chunks per (pass, block),
    padded to the max over cores so one SPMD program fits all)."""
    src = ei[0].astype(np.int64)
    dst = ei[1].astype(np.int64)
    halfs, rows = _node_half_row(src)
    zrow = np.array([ZA, ZB])

    core_edges = []
    counts = np.zeros((NC, 2, NBLK), np.int64)   # per-bucket group counts (padded to 32)
    for c in range(NC):
        m = (dst // NPC) == c
        u, v = src[m], dst[m]
        h, r = halfs[m], rows[m]
        vloc = v - c * NPC
        blk = vloc >> 7
        order = np.lexsort((vloc, blk, h))
        core_edges.append((h[order], r[order], vloc[order], blk[order]))
        # group counts per bucket
        for hh in range(2):
            hm = core_edges[c][0] == hh
            bb = core_edges[c][3][hm]
            vv = core_edges[c][2][hm]
            for b in range(NBLK):
                sel = bb == b
                if not sel.any():
                    counts[c, hh, b] = 0
                    continue
                _, cnt = np.unique(vv[sel], return_counts=True)
                g = ((cnt + GS - 1) // GS).sum()
                counts[c, hh, b] = (g + 31) // 32 * 32
    # chunks per (pass, block) = max over cores
    bc = np.zeros((2, NBLK), np.int64)
    for hh in range(2):
        for b in range(NBLK):
            gmax = counts[:, hh, b].max()
            bc[hh, b] = max(1, (gmax + 127) // 128)   # >=1 chunk per bucket

    per_core = []
    for c in range(NC):
        h_all, r_all, v_all, b_all = core_edges[c]
        pass_slots = [[], []]
        pass_dst = [[], []]
        for hh in range(2):
            hm = h_all == hh
            rr, vv, bb = r_all[hm], v_all[hm], b_all[hm]
            for b in range(NBLK):
                sel = bb == b
                rows_b = rr[sel]
                v_b = vv[sel]
                n_groups_target = bc[hh, b] * 128
                slots = np.full(n_groups_target * GS, zrow[hh], np.int64)
                gdst = np.zeros(n_groups_target, np.int64)
                if sel.any():
                    nodes, cnt = np.unique(v_b, return_counts=True)
                    k4 = (cnt + GS - 1) // GS * GS
                    pad_off = np.concatenate([[0], np.cumsum(k4)[:-1]])
                    orig_off = np.concatenate([[0], np.cumsum(cnt)[:-1]])
                    node_of_edge = np.repeat(np.arange(len(nodes)), cnt)
                    intra = np.arange(len(rows_b)) - orig_off[node_of_edge]
                    slots[pad_off[node_of_edge] + intra] = rows_b
                    ng_nodes = k4 // GS
                    gdst[: ng_nodes.sum()] = np.repeat(nodes & 127, ng_nodes)
                pass_slots[hh].append(slots)
                pass_dst[hh].append(gdst)
        d = {}
        for hh, tag in ((0, "A"), (1, "B")):
            slots = np.concatenate(pass_slots[hh])
            gdst = np.concatenate(pass_dst[hh])
            nchunks = len(gdst) // 128
            # pad chunk count to window multiple for dstloc loads
            cpad = (nchunks + WCH - 1) // WCH * WCH
            gdst = np.concatenate([gdst, np.zeros((cpad - nchunks) * 128, np.int64)])
            slots_pad = np.concatenate(
                [slots, np.full((cpad - nchunks) * CHUNK, zrow[hh], np.int64)])
            d["gidx" + tag] = _wrap_idx(slots_pad)
            d["dstloc" + tag] = (
                gdst.reshape(-1, 128).T.astype(np.float32).copy())  # [128, cpad]
        per_core.append(d)
    meta = {
        "bc": bc,
        "CA": int(bc[0].sum()), "CB": int(bc[1].sum()),
    }
    return per_core, meta


def _prep_head(src, dst):
    """Bucket pairs into (srchalf, dsthalf) quadrants per core."""
    hs_all, rs_all = _node_half_row(src.astype(np.int64))
    hd_all, rd_all = _node_half_row(dst.astype(np.int64))
    per_core_raw = []
    qchunks = np.zeros((NC, 4), np.int64)
    for c in range(NC):
        sl = slice(c * PPC, (c + 1) * PPC)
        hs, rs, hd, rd = hs_all[sl], rs_all[sl], hd_all[sl], rd_all[sl]
        quad = hs * 2 + hd
        order = np.argsort(quad, kind="stable")
        per_core_raw.append((quad[order], rs[order], rd[order], order))
        for q in range(4):
            nq = int((quad == q).sum())
            qchunks[c, q] = (nq + 127) // 128
    qc = qchunks.max(axis=0)
    per_core = []
    zrow = np.array([ZA, ZB])
    for c in range(NC):
        quad, rs, rd, order = per_core_raw[c]
        srows, drows, s2p = [], [], []
        for q in range(4):
            m = quad == q
            nq = int(m.sum())
            tgt = int(qc[q]) * 128
            sr = np.full(tgt, zrow[q // 2], np.int64)
            dr = np.full(tgt, zrow[q % 2], np.int64)
            sp = np.full(tgt, -1, np.int64)
            sr[:nq] = rs[m]
            dr[:nq] = rd[m]
            sp[:nq] = order[m] + c * PPC
            srows.append(sr)
            drows.append(dr)
            s2p.append(sp)
        per_core.append({
            "hsrc": _wrap_idx(np.concatenate(srows)),
            "hdst": _wrap_idx(np.concatenate(drows)),
            "s2p": np.concatenate(s2p),
        })
    return per_core, [int(x) for x in qc]


def _build(meta):
    bc = meta["bc"]
    CA, CB = meta["CA"], meta["CB"]
    qc = meta["qc"]
    CH = sum(qc)
    CA_pad = (CA + WCH - 1) // WCH * WCH
    CB_pad = (CB + WCH - 1) // WCH * WCH

    nc = bacc.Bacc("TRN2", target_bir_lowering=False, debug=False, num_devices=NC,
                   num_swdge_queues=GQ)

    # ---------------- inputs ----------------
    xT = nc.dram_tensor("xT", [IN_DIM, LPC], F32, kind="ExternalInput")
    w1 = nc.dram_tensor("w1", [IN_DIM, HID], F32, kind="ExternalInput")
    t1rep = nc.dram_tensor("t1rep", [128, HID], F32, kind="ExternalInput")
    w2 = nc.dram_tensor("w2", [HID, OUT], F32, kind="ExternalInput")
    t2rep = nc.dram_tensor("t2rep", [128, OUT], F32, kind="ExternalInput")
    wab = nc.dram_tensor("wab", [OUT, 2 * OUT], F32, kind="ExternalInput")
    hb1rep = nc.dram_tensor("hb1rep", [128, 2 * OUT], F32, kind="ExternalInput")
    hw2rep = nc.dram_tensor("hw2rep", [128, OUT], F32, kind="ExternalInput")
    hb2sig = nc.dram_tensor("hb2sig", [128, 1], F32, kind="ExternalInput")
    degp1 = nc.dram_tensor("degp1", [128, NBLK], F32, kind="ExternalInput")
    gidxA = nc.dram_tensor("gidxA", [128, CA_pad * CHUNK // 16], I16, kind="ExternalInput")
    gidxB = nc.dram_tensor("gidxB", [128, CB_pad * CHUNK // 16], I16, kind="ExternalInput")
    dstlocA = nc.dram_tensor("dstlocA", [128, CA_pad], F32, kind="ExternalInput")
    dstlocB = nc.dram_tensor("dstlocB", [128, CB_pad], F32, kind="ExternalInput")
    hsrc = nc.dram_tensor("hsrc", [128, CH * 8], I16, kind="ExternalInput")
    hdst = nc.dram_tensor("hdst", [128, CH * 8], I16, kind="ExternalInput")
    headout = nc.dram_tensor("headout", [128, CH], F32, kind="ExternalOutput")

    CAB = [CA, CB]
    gidx_d = [gidxA, gidxB]
    dstloc_d = [dstlocA, dstlocB]

    with tile.TileContext(nc) as tc:
        with (
            tc.tile_pool(name="res", bufs=1) as res,
            tc.tile_pool(name="dram", bufs=1, space="DRAM") as dram,
        ):
            # ------- internal DRAM: bounce buffers + gather tables -------
            bounce = {}
            tabs = {}
            for li, fdim, dt in ((1, HID, BF16), (2, OUT, F32), (3, 2 * OUT, BF16)):
                bounce[(li, 0)] = dram.tile([HPA, fdim], dt, name=f"bnc{li}a")
                bounce[(li, 1)] = dram.tile([HPB, fdim], dt, name=f"bnc{li}b")
                tabs[(li, 0)] = dram.tile([VA, fdim], dt, name=f"tab{li}a",
                                          addr_space="Shared")
                tabs[(li, 1)] = dram.tile([VB, fdim], dt, name=f"tab{li}b",
                                          addr_space="Shared")

            # ------- constants -------
            ident = res.tile([128, 128], F32)
            from concourse.masks import make_identity
            make_identity(nc, ident[:])
            colI = res.tile([128, 128], mybir.dt.int32)
            nc.gpsimd.iota(colI[:], pattern=[[1, 128]], base=0, channel_multiplier=0)
            colf = res.tile([128, 128], F32)
            nc.vector.tensor_copy(colf[:], colI[:])
            # bd[p, g] = 1 if p//GS == g
            pI = res.tile([128, 32], mybir.dt.int32)
            nc.gpsimd.iota(pI[:], pattern=[[0, 32]], base=0, channel_multiplier=1)
            pf = res.tile([128, 32], F32)
            nc.vector.tensor_copy(pf[:], pI[:])
            g4 = res.tile([128, 32], F32)
            nc.vector.tensor_scalar(out=g4[:], in0=colf[:, :32], scalar1=float(GS),
                                    scalar2=None, op0=mybir.AluOpType.mult)
            lo = res.tile([128, 32], F32)
            nc.vector.tensor_tensor(out=lo[:], in0=pf[:], in1=g4[:], op=mybir.AluOpType.is_ge)
            hi0 = res.tile([128, 32], F32)
            nc.vector.tensor_scalar(out=hi0[:], in0=g4[:], scalar1=float(GS - 1),
                                    scalar2=None, op0=mybir.AluOpType.add)
            hi = res.tile([128, 32], F32)
            nc.vector.tensor_tensor(out=hi[:], in0=pf[:], in1=hi0[:], op=mybir.AluOpType.is_le)
            bdf = res.tile([128, 32], F32)
            nc.vector.tensor_tensor(out=bdf[:], in0=lo[:], in1=hi[:], op=mybir.AluOpType.mult)
            bdb = res.tile([128, 32], BF16)
            nc.vector.tensor_copy(bdb[:], bdf[:])

            # zero rows of the four xsc tables
            zf = res.tile([128, HID], F32)
            nc.gpsimd.memset(zf[:], 0.0)
            zb = res.tile([128, HID], BF16)
            nc.gpsimd.memset(zb[:], 0.0)
            for li, fdim, zt in ((1, HID, zb), (2, OUT, zf), (3, 2 * OUT, zb)):
                nc.sync.dma_start(bounce[(li, 0)][HLOC_A:HPA, :], zt[:16, :fdim])
                nc.sync.dma_start(bounce[(li, 1)][HLOC_B:HPB, :], zt[:16, :fdim])

            # dinv = 1/sqrt(deg+1); dinv2 = 1/(deg+1)
            dinv2 = res.tile([128, NBLK], F32)
            dp1 = res.tile([128, NBLK], F32)
            nc.sync.dma_start(dp1[:], degp1[:, :])
            nc.vector.reciprocal(dinv2[:], dp1[:])
            dinv = res.tile([128, NBLK], F32)
            nc.scalar.sqrt(dinv[:], dinv2[:])

            # small weights
            w1t = res.tile([128, 4, HID], F32)
            nc.sync.dma_start(w1t[:], w1[:, :].rearrange("(c p) h -> p c h", p=128))
            w2t = res.tile([128, OUT], F32)
            nc.sync.dma_start(w2t[:], w2[:, :])
            wabt = res.tile([OUT, 2 * OUT], F32)
            nc.sync.dma_start(wabt[:], wab[:, :])
            t1r = res.tile([128, HID], F32)
            nc.sync.dma_start(t1r[:], t1rep[:, :])
            t2r = res.tile([128, OUT], F32)
            nc.sync.dma_start(t2r[:], t2rep[:, :])
            hb1r = res.tile([128, 2 * OUT], F32)
            nc.sync.dma_start(hb1r[:], hb1rep[:, :])
            hw2r = res.tile([128, OUT], F32)
            nc.sync.dma_start(hw2r[:], hw2rep[:, :])
            hb2t = res.tile([128, 1], F32)
            nc.sync.dma_start(hb2t[:], hb2sig[:, :])

            # resident activations
            xw1 = res.tile([128, NBLK * HID], F32)
            hres = res.tile([128, NBLK * HID], F32)
            partials = res.tile([128, NBLK * HID], F32)
            xw2 = res.tile([128, NBLK * OUT], F32)
            zres = res.tile([128, NBLK * OUT], F32)

            # ---------------- phase A: xw1 = xT.T @ (W1*s1) ----------------
            with (
                tc.tile_pool(name="pha", bufs=1) as pha,
                tc.tile_pool(name="phaps", bufs=4, space="PSUM") as phaps,
                tc.tile_pool(name="evp", bufs=4) as evp,
            ):
                xtt = [pha.tile([128, LPC], F32, tag=f"xt{c}", name=f"xt{c}")
                       for c in range(4)]
                for c in range(4):
                    nc.sync.dma_start(xtt[c][:], xT[c * 128:(c + 1) * 128, :])
                for b in range(NBLK):
                    ps = phaps.tile([128, HID], F32, tag="ps", space="PSUM")
                    for c in range(4):
                        nc.tensor.matmul(ps[:, :], lhsT=xtt[c][:, b * 128:(b + 1) * 128],
                                         rhs=w1t[:, c, :], start=(c == 0), stop=(c == 3))
                    nc.scalar.copy(xw1[:, b * HID:(b + 1) * HID], ps[:, :])
                    ev = evp.tile([128, HID], BF16, tag="ev1")
                    nc.vector.tensor_scalar(out=ev[:], in0=xw1[:, b * HID:(b + 1) * HID],
                                            scalar1=dinv[:, b:b + 1], scalar2=None,
                                            op0=mybir.AluOpType.mult)
                    if b < NBLK_A:
                        nc.sync.dma_start(bounce[(1, 0)][b * 128:(b + 1) * 128, :], ev[:])
                    else:
                        bb = b - NBLK_A
                        nc.sync.dma_start(bounce[(1, 1)][bb * 128:(bb + 1) * 128, :], ev[:])

            for hh in range(2):
                nc.gpsimd.collective_compute(
                    "AllGather", mybir.AluOpType.bypass,
                    replica_groups=[list(range(NC))],
                    ins=[bounce[(1, hh)].opt()],
                    outs=[tabs[(1, hh)].opt()],
                )

            # ---------------- aggregation (shared for both layers) ----------------
            def aggregate(layer, fdim, tabdt, out_cb):
                """Run both passes of edge aggregation for `layer`.
                out_cb(b, blockpsum_ap, passidx) consumes the final block psum."""
                qn = [0]
                with (
                    tc.tile_pool(name=f"ag{layer}", bufs=3) as wp,
                    tc.tile_pool(name=f"ag{layer}i", bufs=3) as ip,
                    tc.tile_pool(name=f"ag{layer}ps", bufs=2, space="PSUM") as pp,
                ):
                    for hh in range(2):
                        Cp = CAB[hh]
                        wins = {}

                        def make_window(w, hh=hh):
                            nch = min(WCH, Cp - w * WCH)
                            idx_t = ip.tile([128, WCH * CHUNK // 16], I16, tag="idx",
                                            name="idxw")
                            nc.sync.dma_start(
                                idx_t[:, :nch * CHUNK // 16],
                                gidx_d[hh][:, w * WCH * CHUNK // 16:
                                           (w * WCH + nch) * CHUNK // 16])
                            msg = wp.tile([128, WCH * GS, fdim], tabdt, tag="msg",
                                          name="msgw")
                            nc.gpsimd.dma_gather(
                                out_ap=msg[:, :nch * GS, :],
                                in_ap=tabs[(layer, hh)][:, :],
                                idxs_ap=idx_t[:, :nch * CHUNK // 16],
                                num_idxs=nch * CHUNK, num_idxs_reg=nch * CHUNK,
                                elem_size=fdim, single_packet=False,
                                queue_num=qn[0] % GQ)
                            qn[0] += 1
                            dl = ip.tile([128, WCH], F32, tag="dl", name="dlw")
                            nc.sync.dma_start(dl[:], dstloc_d[hh][:, w * WCH:(w + 1) * WCH])
                            oh = wp.tile([128, WCH * 128], tabdt, tag="oh", name="ohw")
                            nc.vector.tensor_tensor(
                                out=oh[:].rearrange("p (c j) -> p c j", j=128),
                                in0=dl[:, :, None].broadcast_to([128, WCH, 128]),
                                in1=colf[:, None, :].broadcast_to([128, WCH, 128]),
                                op=mybir.AluOpType.is_equal)
                            wins[w] = (msg, oh)

                        # chunks
                        kk = 0
                        for b in range(NBLK):
                            nch_b = int(bc[hh][b])
                            bp = pp.tile([128, fdim], F32, tag="blk", space="PSUM",
                                         bufs=2, name="bp")
                            for j in range(nch_b):
                                w, m = kk // WCH, kk % WCH
                                if w not in wins:
                                    make_window(w)
                                msg, oh = wins[w]
                                l1 = pp.tile([128, fdim], F32, tag="l1", space="PSUM",
                                             bufs=3, name="l1")
                                for jj in range(4):
                                    nc.tensor.matmul(
                                        l1[32 * jj:32 * (jj + 1), :],
                                        lhsT=(bdb if tabdt == BF16 else bdf)[:, :],
                                        rhs=msg[:, m * GS + jj, :],
                                        start=True, stop=True,
                                        tile_position=(0, 32 * jj))
                                stg = wp.tile([128, fdim], tabdt, tag="stg")
                                nc.scalar.copy(stg[:], l1[:])
                                nc.tensor.matmul(
                                    bp[:, :], lhsT=oh[:, m * 128:(m + 1) * 128],
                                    rhs=stg[:, :],
                                    start=(j == 0), stop=(j == nch_b - 1))
                                kk += 1
                            out_cb(b, bp, hh, wp)
                # end pools

            # ---- layer 1 aggregation ----
            def l1_out(b, bp, hh, wp):
                if hh == 0:
                    nc.scalar.copy(partials[:, b * HID:(b + 1) * HID], bp[:, :])
                else:
                    t = wp.tile([128, HID], F32, tag="po1")
                    nc.vector.tensor_tensor(out=t[:], in0=bp[:, :],
                                            in1=partials[:, b * HID:(b + 1) * HID],
                                            op=mybir.AluOpType.add)
                    y = wp.tile([128, HID], F32, tag="po2")
                    nc.scalar.activation(y[:], t[:], mybir.ActivationFunctionType.Copy,
                                         bias=0.0, scale=dinv[:, b:b + 1])
                    t2 = wp.tile([128, HID], F32, tag="po3")
                    nc.vector.tensor_scalar(out=t2[:], in0=xw1[:, b * HID:(b + 1) * HID],
                                            scalar1=dinv2[:, b:b + 1], scalar2=None,
                                            op0=mybir.AluOpType.mult)
                    t3 = wp.tile([128, HID], F32, tag="po4")
                    nc.vector.tensor_tensor(out=t3[:], in0=y[:], in1=t2[:],
                                            op=mybir.AluOpType.add)
                    t4 = wp.tile([128, HID], F32, tag="po5")
                    nc.vector.tensor_tensor(out=t4[:], in0=t3[:], in1=t1r[:],
                                            op=mybir.AluOpType.add)
                    nc.scalar.activation(hres[:, b * HID:(b + 1) * HID], t4[:],
                                         mybir.ActivationFunctionType.Relu)

            aggregate(1, HID, BF16, l1_out)

            # ---------------- layer-2 projection: xw2 = h @ (W2*s2) ----------------
            with (
                tc.tile_pool(name="l2p", bufs=4) as l2p,
                tc.tile_pool(name="l2ps", bufs=4, space="PSUM") as l2ps,
            ):
                for b in range(NBLK):
                    tp = l2ps.tile([128, 128], F32, tag="tp", space="PSUM")
                    nc.tensor.transpose(tp[:, :], hres[:, b * HID:(b + 1) * HID], ident[:])
                    ht = l2p.tile([128, 128], F32, tag="ht")
                    nc.scalar.copy(ht[:], tp[:])
                    ps2 = l2ps.tile([128, OUT], F32, tag="ps2", space="PSUM")
                    nc.tensor.matmul(ps2[:, :], lhsT=ht[:, :], rhs=w2t[:, :],
                                     start=True, stop=True)
                    nc.scalar.copy(xw2[:, b * OUT:(b + 1) * OUT], ps2[:, :])
                    ev = l2p.tile([128, OUT], F32, tag="ev2")
                    nc.vector.tensor_scalar(out=ev[:], in0=xw2[:, b * OUT:(b + 1) * OUT],
                                            scalar1=dinv[:, b:b + 1], scalar2=None,
                                            op0=mybir.AluOpType.mult)
                    if b < NBLK_A:
                        nc.sync.dma_start(bounce[(2, 0)][b * 128:(b + 1) * 128, :], ev[:])
                    else:
                        bb = b - NBLK_A
                        nc.sync.dma_start(bounce[(2, 1)][bb * 128:(bb + 1) * 128, :], ev[:])
            for hh in range(2):
                nc.gpsimd.collective_compute(
                    "AllGather", mybir.AluOpType.bypass,
                    replica_groups=[list(range(NC))],
                    ins=[bounce[(2, hh)].opt()],
                    outs=[tabs[(2, hh)].opt()],
                )

            # ---- layer 2 aggregation ----
            def l2_out(b, bp, hh, wp):
                if hh == 0:
                    nc.scalar.copy(partials[:, b * OUT:(b + 1) * OUT], bp[:, :])
                else:
                    t = wp.tile([128, OUT], F32, tag="po1")
                    nc.vector.tensor_tensor(out=t[:], in0=bp[:, :],
                                            in1=partials[:, b * OUT:(b + 1) * OUT],
                                            op=mybir.AluOpType.add)
                    y = wp.tile([128, OUT], F32, tag="po2")
                    nc.scalar.activation(y[:], t[:], mybir.ActivationFunctionType.Copy,
                                         bias=0.0, scale=dinv[:, b:b + 1])
                    t2 = wp.tile([128, OUT], F32, tag="po3")
                    nc.vector.tensor_scalar(out=t2[:], in0=xw2[:, b * OUT:(b + 1) * OUT],
                                            scalar1=dinv2[:, b:b + 1], scalar2=None,
                                            op0=mybir.AluOpType.mult)
                    t3 = wp.tile([128, OUT], F32, tag="po4")
                    nc.vector.tensor_tensor(out=t3[:], in0=y[:], in1=t2[:],
                                            op=mybir.AluOpType.add)
                    t4 = wp.tile([128, OUT], F32, tag="po5")
                    nc.vector.tensor_tensor(out=t4[:], in0=t3[:], in1=t2r[:],
                                            op=mybir.AluOpType.add)
                    nc.scalar.activation(zres[:, b * OUT:(b + 1) * OUT], t4[:],
                                         mybir.ActivationFunctionType.Relu)

            aggregate(2, OUT, F32, l2_out)

            # ---------------- AB projection ----------------
            with (
                tc.tile_pool(name="abp", bufs=4) as abp,
                tc.tile_pool(name="abps", bufs=4, space="PSUM") as abps,
            ):
                for b in range(NBLK):
                    tp = abps.tile([128, 128], F32, tag="tp", space="PSUM")
                    nc.tensor.transpose(tp[:OUT, :], zres[:, b * OUT:(b + 1) * OUT],
                                        ident[:])
                    zt = abp.tile([OUT, 128], F32, tag="zt")
                    nc.scalar.copy(zt[:], tp[:OUT, :])
                    psb = abps.tile([128, 2 * OUT], F32, tag="psb", space="PSUM")
                    nc.tensor.matmul(psb[:, :], lhsT=zt[:, :], rhs=wabt[:, :],
                                     start=True, stop=True)
                    ev = abp.tile([128, 2 * OUT], BF16, tag="ev3")
                    nc.vector.tensor_tensor(out=ev[:], in0=psb[:, :], in1=hb1r[:],
                                            op=mybir.AluOpType.add)
                    if b < NBLK_A:
                        nc.sync.dma_start(bounce[(3, 0)][b * 128:(b + 1) * 128, :], ev[:])
                    else:
                        bb = b - NBLK_A
                        nc.sync.dma_start(bounce[(3, 1)][bb * 128:(bb + 1) * 128, :], ev[:])
            for hh in range(2):
                nc.gpsimd.collective_compute(
                    "AllGather", mybir.AluOpType.bypass,
                    replica_groups=[list(range(NC))],
                    ins=[bounce[(3, hh)].opt()],
                    outs=[tabs[(3, hh)].opt()],
                )

            # ---------------- head ----------------
            qc = meta["qc"]
            outstage = res.tile([128, CH], F32)
            with (
                tc.tile_pool(name="hd", bufs=2) as hd,
                tc.tile_pool(name="hdi", bufs=3) as hdi,
            ):
                qn = 0
                cbase = 0
                for q in range(4):
                    hs, hdh = q // 2, q % 2
                    nchq = qc[q]
                    for w0 in range(0, nchq, HWCH):
                        nch = min(HWCH, nchq - w0)
                        c0 = cbase + w0
                        sidx = hdi.tile([128, HWCH * 8], I16, tag="sidx")
                        nc.sync.dma_start(sidx[:, :nch * 8],
                                          hsrc[:, c0 * 8:(c0 + nch) * 8])
                        didx = hdi.tile([128, HWCH * 8], I16, tag="didx")
                        nc.sync.dma_start(didx[:, :nch * 8],
                                          hdst[:, c0 * 8:(c0 + nch) * 8])
                        ms = hd.tile([128, HWCH, 2 * OUT], BF16, tag="ms")
                        nc.gpsimd.dma_gather(
                            out_ap=ms[:, :nch, :], in_ap=tabs[(3, hs)][:, :],
                            idxs_ap=sidx[:, :nch * 8],
                            num_idxs=nch * 128, num_idxs_reg=nch * 128,
                            elem_size=2 * OUT, single_packet=False,
                            queue_num=qn % GQ)
                        qn += 1
                        md = hd.tile([128, HWCH, 2 * OUT], BF16, tag="md")
                        nc.gpsimd.dma_gather(
                            out_ap=md[:, :nch, :], in_ap=tabs[(3, hdh)][:, :],
                            idxs_ap=didx[:, :nch * 8],
                            num_idxs=nch * 128, num_idxs_reg=nch * 128,
                            elem_size=2 * OUT, single_packet=False,
                            queue_num=qn % GQ)
                        qn += 1
                        s = hd.tile([128, HWCH, OUT], F32, tag="s")
                        nc.vector.tensor_tensor(out=s[:, :nch, :],
                                                in0=ms[:, :nch, 0:OUT],
                                                in1=md[:, :nch, OUT:2 * OUT],
                                                op=mybir.AluOpType.add)
                        hm = hd.tile([128, HWCH, OUT], F32, tag="hm")
                        nc.scalar.activation(hm[:, :nch, :], s[:, :nch, :],
                                             mybir.ActivationFunctionType.Relu)
                        t = hd.tile([128, HWCH, OUT], F32, tag="t")
                        nc.vector.tensor_tensor(
                            out=t[:, :nch, :], in0=hm[:, :nch, :],
                            in1=hw2r[:, None, :].broadcast_to([128, nch, OUT]),
                            op=mybir.AluOpType.mult)
                        lg = hd.tile([128, HWCH], F32, tag="lg")
                        nc.vector.tensor_reduce(
                            out=lg[:, :nch], in_=t[:, :nch, :],
                            op=mybir.AluOpType.add, axis=mybir.AxisListType.X)
                        nc.scalar.activation(outstage[:, c0:c0 + nch], lg[:, :nch],
                                             mybir.ActivationFunctionType.Sigmoid,
                                             bias=hb2t[:, 0:1])
                    cbase += nchq
            nc.sync.dma_start(headout[:, :], outstage[:])

    nc.compile()
    return nc


def kernel(x, ei, src, dst, W1, b1, bn1_gamma, bn1_beta, bn1_mean, bn1_var,
           W2, b2, bn2_gamma, bn2_beta, bn2_mean, bn2_var, hW1, hb1, hW2, hb2):
    x = np.asarray(x, np.float32)
    ei = np.asarray(ei)
    src = np.asarray(src)
    dst = np.asarray(dst)

    # ----- fold BN into weights (host param prep) -----
    s1 = (np.asarray(bn1_gamma) / np.sqrt(np.asarray(bn1_var) + EPS)).astype(np.float32)
    t1 = (np.asarray(bn1_beta) - np.asarray(bn1_mean) * s1 + np.asarray(b1) * s1).astype(np.float32)
    s2 = (np.asarray(bn2_gamma) / np.sqrt(np.asarray(bn2_var) + EPS)).astype(np.float32)
    t2 = (np.asarray(bn2_beta) - np.asarray(bn2_mean) * s2 + np.asarray(b2) * s2).astype(np.float32)
    W1f = (np.asarray(W1, np.float32) * s1[None, :])
    W2f = (np.asarray(W2, np.float32) * s2[None, :])
    hW1 = np.asarray(hW1, np.float32)
    Wab = np.concatenate([hW1[:OUT, :], hW1[OUT:, :]], axis=1)  # [64, 128]

    deg = np.bincount(ei[1].astype(np.int64), minlength=N).astype(np.float32) + 1.0

    per_edges, meta = _prep_edges(ei)
    per_head, qc = _prep_head(src, dst)
    meta["qc"] = qc
    CH = sum(qc)

    key = (meta["CA"], meta["CB"], tuple(meta["bc"].ravel()), tuple(qc))
    if key not in _CACHE:
        _CACHE[key] = _build(meta)
    nc = _CACHE[key]

    t1rep = np.tile(t1[None, :], (128, 1)).astype(np.float32)
    t2rep = np.tile(t2[None, :], (128, 1)).astype(np.float32)
    hb1rep = np.tile(np.concatenate([np.asarray(hb1, np.float32),
                                     np.zeros(OUT, np.float32)])[None, :], (128, 1))
    hw2rep = np.tile(np.asarray(hW2, np.float32).reshape(1, OUT), (128, 1))
    hb2sig = np.full((128, 1), float(np.asarray(hb2).reshape(-1)[0]), np.float32)

    in_maps = []
    for c in range(NC):
        xs = np.zeros((LPC, IN_DIM), np.float32)
        xs[:NPC] = x[c * NPC:(c + 1) * NPC]
        dp = np.ones((128, NBLK), np.float32)
        dloc = deg[c * NPC:(c + 1) * NPC]
        dpad = np.concatenate([dloc, np.ones(LPC - NPC, np.float32)])
        dp[:, :] = dpad.reshape(NBLK, 128).T
        m = {
            "xT": np.ascontiguousarray(xs.T),
            "w1": np.ascontiguousarray(W1f),
            "t1rep": t1rep, "w2": np.ascontiguousarray(W2f), "t2rep": t2rep,
            "wab": np.ascontiguousarray(Wab), "hb1rep": hb1rep,
            "hw2rep": hw2rep, "hb2sig": hb2sig,
            "degp1": np.ascontiguousarray(dp),
            "gidxA": per_edges[c]["gidxA"], "gidxB": per_edges[c]["gidxB"],
            "dstlocA": per_edges[c]["dstlocA"], "dstlocB": per_edges[c]["dstlocB"],
            "hsrc": per_head[c]["hsrc"], "hdst": per_head[c]["hdst"],
        }
        in_maps.append(m)

    res = run_bass_kernel_spmd(nc, in_maps, core_ids=list(range(NC)))

    out = np.zeros(P, np.float32)
    for c in range(NC):
        vals = res.results[c]["headout"]          # [128, CH]
        vals = vals.T.reshape(-1)                  # slot s = c*128+p -> pos s
        s2p = per_head[c]["s2p"]
        mvalid = s2p >= 0
        out[s2p[mvalid]] = vals[mvalid]
    return out
